# revision 1
# baseline (speedup 1.0000x reference)
"""Trainium2 Bass kernel for nn_BilinearLabelAttention.

out[b,l,i,o] = sum_j head[b,i,j] * label_U_diag[l,j] * dep[b,o,j]
  head/dep: [8, 512, 512] f32, label_U_diag: [32, 512] f32
  out: [8, 32, 512, 512] f32

Sharding: data-parallel over batch — core b computes out[b]. Per core that
is L=32 matmuls of (head*diag(U_l)) @ dep^T, i.e. 512 PE matmuls of
[128j,128i]^T @ [128j,512o] accumulated over 4 j-tiles in PSUM.

Inputs are pre-transposed on the host (headT/depT/uT with j leading) so the
contraction dim lands on the SBUF partition axis without on-device
transposes. Matmuls run in float32r (single-pass fp32, ~1 cycle/row at
N=512) giving ~1.7e-4 max relative error vs the fp32 reference at near-bf16
speed. The per-label diagonal scaling runs on the Vector engine as a
per-partition tensor_scalar multiply; PSUM evacuation runs on the Scalar
engine; outputs stream straight to HBM.
"""

import os

import numpy as np

os.environ.setdefault("BASS_NEVER_TRACE", "1")

import concourse.bass as bass
import concourse.mybir as mybir
from concourse.bass_utils import run_bass_kernel_spmd
from concourse.tile import TileContext
from concourse.vector_clock import ScopedClock

B, S, D, L = 8, 512, 512, 32
P = 128
KT = D // P
MT = S // P


class _LeanTailTileContext(TileContext):
    """TileContext exit without the second all-engine barrier: engines with
    nothing left simply halt; semaphore clears still happen after the
    pre-clear barrier, so repeat executions stay correct."""

    def _drain_and_barrier(self, tick_clock, wait_clock):
        drain_inst = self.nc.sync.drain()
        wait_clock.add_sem_waits(
            drain_inst.ins, ScopedClock({None: tick_clock.global_clock})
        )
        self.nc.all_engine_barrier()
        assert self.sems is not None
        popped = self.nc._tile_sem_poison_stack.pop()
        assert popped is self._sem_poison
        self.nc.clear_and_free_semaphores(list(self.sems.allocated().values()))


def _spread_multi_waits(nc):
    """The walrus build in this container accepts at most ONE semaphore wait
    per instruction ("Too many sync wait commands"). Hoist all-but-one wait
    of each multi-wait instruction onto single-wait NoOps inserted before it
    on the same engine queue (engines execute in order, so gating the queue
    earlier is equivalent)."""
    for f in nc.m.functions:
        for bb in f.blocks:
            new_insts = []
            for ins in bb.instructions:
                w = list(ins.sync_info.on_wait) if ins.sync_info else []
                if len(w) > 1:
                    for extra in w[:-1]:
                        nop = mybir.InstNoOp(
                            name=nc.get_next_instruction_name(), ins=[], outs=[]
                        )
                        nop.engine = ins.engine
                        nop.sync_info = mybir.SyncInfo(on_wait=[extra], on_update=[])
                        new_insts.append(nop)
                    ins.sync_info.on_wait = [w[-1]]
                new_insts.append(ins)
            bb.instructions[:] = new_insts


def _strip_const_memsets(nc):
    """Bass's preamble memsets four const-* SBUF tiles this kernel never
    reads; they run through the GpSimd DGE queue and hold the entry barrier
    behind ~3.5us of cold-queue latency. Drop them."""
    bb = nc.m.functions[0].blocks[0]
    bb.instructions[:] = [
        ins
        for ins in bb.instructions
        if not (
            type(ins).__name__ == "InstMemset"
            and str(ins.engine).endswith("Pool")
            and not ins.sync_info
        )
    ]


def _build():
    f32 = mybir.dt.float32
    f32r = mybir.dt.float32r

    nc = bass.Bass(enable_partition_id=False)
    headT = nc.declare_dram_parameter("headT", [D, S], f32, isOutput=False)
    depT = nc.declare_dram_parameter("depT", [D, S], f32, isOutput=False)
    uT = nc.declare_dram_parameter("uT", [D, L], f32, isOutput=False)
    out = nc.declare_dram_parameter("out", [L, S, S], f32, isOutput=True)

    with _LeanTailTileContext(nc) as tc:
        with (
            tc.tile_pool(name="inputs", bufs=1) as in_pool,
            tc.tile_pool(name="scaled", bufs=12) as sc_pool,
            tc.tile_pool(name="outs", bufs=16) as out_pool,
            tc.tile_pool(name="psum", bufs=8, space="PSUM") as ps_pool,
        ):
            head_sb, dep_sb, u_sb = [], [], []
            H = S // 2
            # Input loads issue on three engine queues in parallel (dep on
            # sync/HWDGE, head on scalar, u on gpsimd) so descriptor
            # generation doesn't serialize; kt ascending so the first
            # label's matmuls can start as soon as kt=0 lands.
            for kt in range(KT):
                d = in_pool.tile([P, S], f32, tag=f"dep{kt}")
                for half in range(2):
                    sl = slice(half * H, (half + 1) * H)
                    nc.sync.dma_start(
                        out=d[:, sl], in_=depT[kt * P : (kt + 1) * P, sl]
                    )
                u = in_pool.tile([P, L], f32, tag=f"u{kt}")
                nc.gpsimd.dma_start(out=u[:], in_=uT[kt * P : (kt + 1) * P, :])
                u_sb.append(u)
                h = in_pool.tile([P, S], f32, tag=f"head{kt}")
                nc.scalar.dma_start(out=h[:], in_=headT[kt * P : (kt + 1) * P, :])
                head_sb.append(h)
                # float32r operands must be produced ("rounded") by a
                # compute engine, not plain DMA; cast in halves so the
                # first half chains right after its DMA.
                dr = in_pool.tile([P, S], f32r, tag=f"depr{kt}")
                for half in range(2):
                    sl = slice(half * H, (half + 1) * H)
                    nc.vector.tensor_copy(out=dr[:, sl], in_=d[:, sl])
                dep_sb.append(dr)

            def make_scaled(l, kt):
                s = sc_pool.tile([P, S], f32r, name=f"s_{l}_{kt}", tag=f"scaled{kt}")
                if l == 0:
                    # Quarter granularity on the first label so the first
                    # matmul waits only on a quarter of head[kt].
                    for mi in range(MT):
                        sl = slice(mi * P, (mi + 1) * P)
                        nc.vector.tensor_scalar_mul(
                            s[:, sl], head_sb[kt][:, sl], u_sb[kt][:, l : l + 1]
                        )
                else:
                    nc.vector.tensor_scalar_mul(
                        s[:], head_sb[kt][:], u_sb[kt][:, l : l + 1]
                    )
                return s

            def evac(l, mi, ps):
                ot = out_pool.tile([P, S], f32, name=f"ot_{l}_{mi}", tag="ot")
                nc.scalar.copy(ot[:], ps[:])
                nc.sync.dma_start(out=out[l, mi * P : (mi + 1) * P, :], in_=ot[:])

            for l in range(L):
                scaled = [make_scaled(l, kt) for kt in range(KT)]
                if l == 0:
                    # kt-outer for the first label: its first matmuls need
                    # only the kt=0 input tiles (which land first).
                    psums = [
                        ps_pool.tile([P, S], f32, name=f"ps_{l}_{mi}", tag="ps")
                        for mi in range(MT)
                    ]
                    for kt in range(KT):
                        for mi in range(MT):
                            nc.tensor.matmul(
                                psums[mi][:],
                                lhsT=scaled[kt][:, mi * P : (mi + 1) * P],
                                rhs=dep_sb[kt][:],
                                start=(kt == 0),
                                stop=(kt == KT - 1),
                            )
                    for mi in range(MT):
                        evac(l, mi, psums[mi])
                    continue
                for mi in range(MT):
                    ps = ps_pool.tile([P, S], f32, name=f"ps_{l}_{mi}", tag="ps")
                    for kt in range(KT):
                        nc.tensor.matmul(
                            ps[:],
                            lhsT=scaled[kt][:, mi * P : (mi + 1) * P],
                            rhs=dep_sb[kt][:],
                            start=(kt == 0),
                            stop=(kt == KT - 1),
                        )
                    evac(l, mi, ps)

    _strip_const_memsets(nc)
    _spread_multi_waits(nc)
    return nc


_NC_CACHE = None


def kernel(head, dep, label_U_diag):
    global _NC_CACHE
    head = np.ascontiguousarray(np.asarray(head, dtype=np.float32))
    dep = np.ascontiguousarray(np.asarray(dep, dtype=np.float32))
    u = np.asarray(label_U_diag, dtype=np.float32)

    uT = np.ascontiguousarray(u.T)  # [D, L]
    in_maps = [
        {
            "headT": np.ascontiguousarray(head[b].T),
            "depT": np.ascontiguousarray(dep[b].T),
            "uT": uT,
        }
        for b in range(B)
    ]

    if _NC_CACHE is None:
        _NC_CACHE = _build()
    res = run_bass_kernel_spmd(_NC_CACHE, in_maps, list(range(B)), trace=False)
    return np.stack([res.results[b]["out"] for b in range(B)])


# revision 3
# speedup vs baseline: 1.0032x; 1.0032x over previous
"""Trainium2 Bass kernel for nn_BilinearLabelAttention.

out[b,l,i,o] = sum_j head[b,i,j] * label_U_diag[l,j] * dep[b,o,j]
  head/dep: [8, 512, 512] f32, label_U_diag: [32, 512] f32
  out: [8, 32, 512, 512] f32

Sharding: data-parallel over batch — core b computes out[b]. Per core that
is L=32 matmuls of (head*diag(U_l)) @ dep^T, i.e. 512 PE matmuls of
[128j,128i]^T @ [128j,512o] accumulated over 4 j-tiles in PSUM.

Inputs are pre-transposed on the host (headT/depT/uT with j leading) so the
contraction dim lands on the SBUF partition axis without on-device
transposes. Matmuls run in float32r (single-pass fp32, ~1 cycle/row at
N=512) giving ~1.7e-4 max relative error vs the fp32 reference at near-bf16
speed. The per-label diagonal scaling runs on the Vector engine as a
per-partition tensor_scalar multiply; PSUM evacuation runs on the Scalar
engine; outputs stream straight to HBM.
"""

import os

import numpy as np

os.environ.setdefault("BASS_NEVER_TRACE", "1")

import concourse.bass as bass
import concourse.mybir as mybir
from concourse.bass_utils import run_bass_kernel_spmd
from concourse.tile import TileContext
from concourse.vector_clock import ScopedClock

B, S, D, L = 8, 512, 512, 32
P = 128
KT = D // P
MT = S // P


class _LeanTailTileContext(TileContext):
    """TileContext exit without the second all-engine barrier: engines with
    nothing left simply halt; semaphore clears still happen after the
    pre-clear barrier, so repeat executions stay correct."""

    def _drain_and_barrier(self, tick_clock, wait_clock):
        drain_inst = self.nc.sync.drain()
        wait_clock.add_sem_waits(
            drain_inst.ins, ScopedClock({None: tick_clock.global_clock})
        )
        self.nc.all_engine_barrier()
        assert self.sems is not None
        popped = self.nc._tile_sem_poison_stack.pop()
        assert popped is self._sem_poison
        self.nc.clear_and_free_semaphores(list(self.sems.allocated().values()))


def _spread_multi_waits(nc):
    """The walrus build in this container accepts at most ONE semaphore wait
    per instruction ("Too many sync wait commands"). Hoist all-but-one wait
    of each multi-wait instruction onto single-wait NoOps inserted before it
    on the same engine queue (engines execute in order, so gating the queue
    earlier is equivalent)."""
    for f in nc.m.functions:
        for bb in f.blocks:
            new_insts = []
            for ins in bb.instructions:
                w = list(ins.sync_info.on_wait) if ins.sync_info else []
                if len(w) > 1:
                    for extra in w[:-1]:
                        nop = mybir.InstNoOp(
                            name=nc.get_next_instruction_name(), ins=[], outs=[]
                        )
                        nop.engine = ins.engine
                        nop.sync_info = mybir.SyncInfo(on_wait=[extra], on_update=[])
                        new_insts.append(nop)
                    ins.sync_info.on_wait = [w[-1]]
                new_insts.append(ins)
            bb.instructions[:] = new_insts


def _strip_const_memsets(nc):
    """Bass's preamble memsets four const-* SBUF tiles this kernel never
    reads; they run through the GpSimd DGE queue and hold the entry barrier
    behind ~3.5us of cold-queue latency. Drop them."""
    bb = nc.m.functions[0].blocks[0]
    bb.instructions[:] = [
        ins
        for ins in bb.instructions
        if not (
            type(ins).__name__ == "InstMemset"
            and str(ins.engine).endswith("Pool")
            and not ins.sync_info
        )
    ]


def _build():
    f32 = mybir.dt.float32
    f32r = mybir.dt.float32r

    nc = bass.Bass(enable_partition_id=False)
    headT = nc.declare_dram_parameter("headT", [D, S], f32, isOutput=False)
    depT = nc.declare_dram_parameter("depT", [D, S], f32, isOutput=False)
    uT = nc.declare_dram_parameter("uT", [D, L], f32, isOutput=False)
    out = nc.declare_dram_parameter("out", [L, S, S], f32, isOutput=True)

    with _LeanTailTileContext(nc) as tc:
        with (
            tc.tile_pool(name="inputs", bufs=1) as in_pool,
            tc.tile_pool(name="scaled", bufs=12) as sc_pool,
            tc.tile_pool(name="outs", bufs=16) as out_pool,
            tc.tile_pool(name="psum", bufs=8, space="PSUM") as ps_pool,
        ):
            # Hybrid input loads: kt0 and kt1 as separate small DMAs (they
            # gate the first matmuls), kt2-3 batched into one strided DMA
            # to keep descriptor-gen short. dep on sync/HWDGE, head on
            # scalar, u on gpsimd — the three queues issue in parallel.
            def load_tensor(dram, eng, tagp):
                t0_ = in_pool.tile([P, S], f32, name=f"{tagp}0", tag=f"{tagp}0")
                eng.dma_start(out=t0_[:], in_=dram[0:P, :])
                t1_ = in_pool.tile([P, S], f32, name=f"{tagp}1", tag=f"{tagp}1")
                eng.dma_start(out=t1_[:], in_=dram[P : 2 * P, :])
                t23 = in_pool.tile([P, 2 * S], f32, name=f"{tagp}23", tag=f"{tagp}23")
                eng.dma_start(
                    out=t23[:].rearrange("p (kt o) -> p kt o", kt=2),
                    in_=dram[2 * P : 4 * P, :].rearrange("(kt p) o -> p kt o", p=P),
                )
                return [t0_[:], t1_[:], t23[:, :S], t23[:, S:]]

            dep_raw = load_tensor(depT, nc.sync, "dep")
            u_all = in_pool.tile([P, KT * L], f32, name="u_all", tag="u_all")
            nc.gpsimd.dma_start(
                out=u_all[:].rearrange("p (kt l) -> p kt l", kt=KT),
                in_=uT.rearrange("(kt p) l -> p kt l", p=P),
            )
            u_sb = [u_all[:, kt * L : (kt + 1) * L] for kt in range(KT)]
            head_sb = load_tensor(headT, nc.scalar, "head")

            dep_sb = []
            for kt in range(KT):
                # float32r operands must be produced ("rounded") by a
                # compute engine, not plain DMA.
                dr = in_pool.tile([P, S], f32r, name=f"depr{kt}", tag=f"depr{kt}")
                nc.vector.tensor_copy(out=dr[:], in_=dep_raw[kt])
                dep_sb.append(dr)

            def make_scaled(l, kt):
                s = sc_pool.tile([P, S], f32r, name=f"s_{l}_{kt}", tag=f"scaled{kt}")
                if l == 0:
                    # Quarter granularity on the first label so the first
                    # matmul waits only on a quarter of head[kt].
                    for mi in range(MT):
                        sl = slice(mi * P, (mi + 1) * P)
                        nc.vector.tensor_scalar_mul(
                            s[:, sl], head_sb[kt][:, sl], u_sb[kt][:, l : l + 1]
                        )
                else:
                    nc.vector.tensor_scalar_mul(
                        s[:], head_sb[kt][:], u_sb[kt][:, l : l + 1]
                    )
                return s

            def evac(l, mi, ps):
                ot = out_pool.tile([P, S], f32, name=f"ot_{l}_{mi}", tag="ot")
                if l >= L - 2 and mi % 2 == 1:
                    # Tail labels alternate ACT/DVE so the final
                    # evacuation chain halves in latency.
                    nc.vector.tensor_copy(out=ot[:], in_=ps[:])
                else:
                    nc.scalar.copy(ot[:], ps[:])
                nc.sync.dma_start(out=out[l, mi * P : (mi + 1) * P, :], in_=ot[:])

            for l in range(L):
                scaled = [make_scaled(l, kt) for kt in range(KT)]
                if l == 0:
                    # kt-outer for the first label: its first matmuls need
                    # only the kt=0 input tiles (which land first).
                    psums = [
                        ps_pool.tile([P, S], f32, name=f"ps_{l}_{mi}", tag="ps")
                        for mi in range(MT)
                    ]
                    for kt in range(KT):
                        for mi in range(MT):
                            nc.tensor.matmul(
                                psums[mi][:],
                                lhsT=scaled[kt][:, mi * P : (mi + 1) * P],
                                rhs=dep_sb[kt][:],
                                start=(kt == 0),
                                stop=(kt == KT - 1),
                            )
                    for mi in range(MT):
                        evac(l, mi, psums[mi])
                    continue
                for mi in range(MT):
                    ps = ps_pool.tile([P, S], f32, name=f"ps_{l}_{mi}", tag="ps")
                    for kt in range(KT):
                        nc.tensor.matmul(
                            ps[:],
                            lhsT=scaled[kt][:, mi * P : (mi + 1) * P],
                            rhs=dep_sb[kt][:],
                            start=(kt == 0),
                            stop=(kt == KT - 1),
                        )
                    evac(l, mi, ps)

    _strip_const_memsets(nc)
    _spread_multi_waits(nc)
    return nc


_NC_CACHE = None


def kernel(head, dep, label_U_diag):
    global _NC_CACHE
    head = np.ascontiguousarray(np.asarray(head, dtype=np.float32))
    dep = np.ascontiguousarray(np.asarray(dep, dtype=np.float32))
    u = np.asarray(label_U_diag, dtype=np.float32)

    uT = np.ascontiguousarray(u.T)  # [D, L]
    in_maps = [
        {
            "headT": np.ascontiguousarray(head[b].T),
            "depT": np.ascontiguousarray(dep[b].T),
            "uT": uT,
        }
        for b in range(B)
    ]

    if _NC_CACHE is None:
        _NC_CACHE = _build()
    res = run_bass_kernel_spmd(_NC_CACHE, in_maps, list(range(B)), trace=False)
    return np.stack([res.results[b]["out"] for b in range(B)])
